# revision 18
# baseline (speedup 1.0000x reference)
"""FP8 GEMM kernel for Trainium2 (8 NeuronCores, SPMD data-parallel over tokens).

Computes: out = fp16( fp32( e5m2(x) @ e4m3(weight.T) ) + bias )
  x      [4, 4096, 4096] fp16
  weight [4096, 4096]    fp16  (out_features, in_features)
  bias   [4096]          fp16
  out    [4, 4096, 4096] fp16

Sharding: token dim (B*S = 16384) split across 8 cores (2048 rows each);
weight + bias replicated. No collectives; host concatenates the outputs.

Layout: the host pre-packs both operands into per-tile K-major blocks
(`[tile][ki=128][ko=32][free]`), so every device load is one fully
contiguous 1-2MB DMA at full rate (XBAR transposes cap at ~190 GB/s and
corrupt data when issued concurrently from two HWDGE queues; K-major
strided reads only manage ~110-200 GB/s due to short bursts).

Per-core kernel:
 - fp16 -> fp8 quantization happens *inside* the load DMAs: SWDGE (gpsimd)
   descriptors cast in-flight (bit-exact RNE, verified vs ml_dtypes), so
   there is no fp16 staging and no compute-engine cast work at all.
 - DoubleRow fp8 matmuls (K=256/instr, moving free dim 2x512) accumulate
   fp32 into PSUM; n-tile outer / m-tile inner loop keeps all of x8
   resident (64KB/part) while w8 n-tiles stream through a 3-deep pool, so
   the PE starts after the first ~3MB of DMA instead of the whole 33MB.
 - Bias add fused into the PSUM eviction on DVE (its only job); output
   stores + bias broadcast go out on the sync HWDGE queue.
"""

import sys

if "/opt/trn_rl_repo" not in sys.path:
    sys.path.insert(0, "/opt/trn_rl_repo")

import numpy as np

B, S, DIN, DOUT = 4, 4096, 4096, 4096
NCORES = 8
M_TOTAL = B * S              # 16384
M_LOC = M_TOTAL // NCORES    # 2048
P = 128
M_TILES = M_LOC // P         # 16 m-tiles of 128 rows
N_TILE = 512
N_TILES = DOUT // N_TILE     # 8
K_SUB = DIN // P             # 32 k-subtiles of 128
K_CHUNKS = K_SUB // 2        # 16 DoubleRow chunks of 256

_cached_nc = None


def _build():
    global _cached_nc
    if _cached_nc is not None:
        return _cached_nc

    import concourse.mybir as mybir
    import concourse.tile as tile
    from concourse import bacc

    nc = bacc.Bacc("TRN2", target_bir_lowering=False, debug=False,
                   num_devices=NCORES)

    # host-packed K-major tile blocks (see make_in_maps)
    xd = nc.dram_tensor("xd", [M_TILES, P, K_SUB, P], mybir.dt.float16,
                        kind="ExternalInput")
    wd = nc.dram_tensor("wd", [N_TILES, P, K_SUB, N_TILE], mybir.dt.float16,
                        kind="ExternalInput")
    bvec = nc.dram_tensor("bvec", [DOUT], mybir.dt.float16,
                          kind="ExternalInput")
    out = nc.dram_tensor("out", [M_LOC, DOUT], mybir.dt.float16,
                         kind="ExternalOutput")

    with tile.TileContext(nc) as tc:
        with tc.tile_pool(name="w8p", bufs=3) as w8p, \
             tc.tile_pool(name="x8p", bufs=1) as x8p, \
             tc.tile_pool(name="boot", bufs=2) as bootp, \
             tc.tile_pool(name="outp", bufs=8) as outp, \
             tc.tile_pool(name="cst", bufs=1) as cst, \
             tc.tile_pool(name="psum", bufs=4, space="PSUM") as psump:

            # bias replicated across the 128 partitions (HWDGE broadcast)
            bias_rep = cst.tile([P, DOUT], mybir.dt.float16)
            nc.sync.dma_start(bias_rep[:],
                              bvec.ap()[None, :].to_broadcast((P, DOUT)))

            # resident fp8 x: 16 tiles of [ki, ko, 128] e5m2
            x8 = [x8p.tile([P, K_SUB, P], mybir.dt.float8e5,
                           tag=f"x8_{m}", name=f"x8_{m}")
                  for m in range(M_TILES)]

            w8 = {}

            def load_w(j, chunks=1):
                # chunks>1 splits along ko into parallel SWDGE cast-DMAs
                # (contiguous 8KB+ runs) to cut first-delivery latency
                w8[j] = w8p.tile([P, K_SUB, N_TILE], mybir.dt.float8e4,
                                 tag="w8", name=f"w8_{j}")
                step = K_SUB // chunks
                for c in range(chunks):
                    ko = slice(c * step, (c + 1) * step)
                    nc.gpsimd.dma_start(w8[j][:, ko, :], wd[j, :, ko, :])

            def load_x(m, chunks=1):
                step = K_SUB // chunks
                for c in range(chunks):
                    ko = slice(c * step, (c + 1) * step)
                    nc.gpsimd.dma_start(x8[m][:, ko, :], xd[m, :, ko, :])

            # ---- bootstrap: the first tiles come in as fp16 over the two
            # fast HWDGE queues + DVE casts, so the PE starts ~10us earlier
            # than the ~210 GB/s SWDGE cast stream allows; everything else
            # streams through SWDGE cast-DMAs concurrently ----
            w8[0] = w8p.tile([P, K_SUB, N_TILE], mybir.dt.float8e4,
                             tag="w8", name="w8_0")
            KO_B = K_SUB // 4
            for c in range(4):
                ko = slice(c * KO_B, (c + 1) * KO_B)
                wst = bootp.tile([P, KO_B, N_TILE], mybir.dt.float16,
                                 tag="wboot", name=f"wboot_{c}", bufs=4)
                nc.sync.dma_start(wst[:], wd[0, :, ko, :])
                nc.vector.tensor_copy(w8[0][:, ko, :], wst[:])
            for m in range(2):
                xst = bootp.tile([P, K_SUB, P], mybir.dt.float16,
                                 tag="xboot", name=f"xboot_{m}")
                nc.scalar.dma_start(xst[:], xd[m])
                nc.vector.tensor_copy(x8[m][:], xst[:])

            load_w(1, chunks=2)
            for m in range(2, 6):
                load_x(m)

            # ---- matmul loop: n-tile outer, m-tile inner; remaining loads
            # interleaved so queue FIFOs match first-use order ----
            for j in range(N_TILES):
                wtile = w8[j]
                for m in range(M_TILES):
                    if j == 0 and m + 6 < M_TILES:
                        load_x(m + 6)
                    if m == 0 and j + 2 < N_TILES:
                        load_w(j + 2)
                    ps = psump.tile([P, N_TILE], mybir.dt.float32, tag="ps",
                                    name=f"ps_{j}_{m}")
                    for kc in range(K_CHUNKS):
                        nc.tensor.matmul(
                            ps[:],
                            x8[m][:, 2 * kc:2 * kc + 2, :],
                            wtile[:, 2 * kc:2 * kc + 2, :],
                            start=(kc == 0),
                            stop=(kc == K_CHUNKS - 1),
                            perf_mode=mybir.MatmulPerfMode.DoubleRow,
                        )
                    ob = outp.tile([P, N_TILE], mybir.dt.float16, tag="ob",
                                   name=f"ob_{j}_{m}")
                    nc.vector.tensor_add(
                        ob[:], ps[:],
                        bias_rep[:, j * N_TILE:(j + 1) * N_TILE])
                    nc.sync.dma_start(
                        out[m * P:(m + 1) * P,
                            j * N_TILE:(j + 1) * N_TILE], ob[:])

    nc.compile()
    _cached_nc = nc
    return nc


def make_in_maps(x, weight, bias):
    x = np.asarray(x)
    weight = np.asarray(weight)
    bias = np.ascontiguousarray(np.asarray(bias))
    assert x.dtype == np.float16 and weight.dtype == np.float16

    # weight [DOUT, DIN] -> [j, ki, ko, n]: wd[j,ki,ko,n] = weight[j*512+n,
    # ko*128+ki] (i.e. weight.T in per-tile K-major blocks)
    wd = np.ascontiguousarray(
        weight.reshape(N_TILES, N_TILE, K_SUB, P).transpose(0, 3, 2, 1))

    xf = x.reshape(M_TOTAL, DIN)
    in_maps = []
    for c in range(NCORES):
        xc = xf[c * M_LOC:(c + 1) * M_LOC]
        # [M_LOC, DIN] -> [m-tile, ki, ko, m]: xd[t,ki,ko,m] = xc[t*128+m,
        # ko*128+ki]
        xd = np.ascontiguousarray(
            xc.reshape(M_TILES, P, K_SUB, P).transpose(0, 3, 2, 1))
        in_maps.append({"xd": xd, "wd": wd, "bvec": bias})
    return in_maps


def gather_out(results):
    out = np.concatenate([r["out"] for r in results], axis=0)
    return out.reshape(B, S, DOUT)


def kernel(x, weight, bias):
    from concourse.bass_utils import run_bass_kernel_spmd

    nc = _build()
    in_maps = make_in_maps(x, weight, bias)
    res = run_bass_kernel_spmd(nc, in_maps, core_ids=list(range(NCORES)))
    return gather_out(res.results)
